# revision 3
# baseline (speedup 1.0000x reference)
"""Trainium2 Bass kernel for nn_AttentionCore64: softmax(Q@K^T)@V (raw exp,
no scaling), B=2 H=16 S=2048 D=64, f32 in/out. B*H sharded over 8 cores.

Design:
- Heads processed in PAIRS: head A operands live on SBUF partitions 0-63,
  head B on 64-127. The two K=64 f32r QK^T matmuls run row-tiled
  (tile_position (0,0) / (64,0)) -> concurrent on the PE array.
- One exp activation covers both heads' 512-wide score chunks (N=1024).
- Software-pipelined emission: next pair's transposes + previous chunk's
  finalize interleave into the current chunk's t-loop, so the ScalarE
  (the bottleneck at ~133us/core) never starves at phase boundaries.
"""

import numpy as np
from contextlib import ExitStack

import concourse.tile as tile
import concourse.mybir as mybir
from concourse import bacc
from concourse.bass_utils import run_bass_kernel_spmd
from concourse.masks import make_identity

B, H, S, D = 2, 16, 2048, 64
NCORES = 8
HPC = (B * H) // NCORES  # 4 heads per core

P = 128
CH = 512            # per-head s-chunk in main loop
NCH = S // CH       # 4 chunk passes per pair
NT = S // P         # 16 key tiles
NO = S // P         # 16 s(o)-tiles
DT = mybir.dt
AF = mybir.ActivationFunctionType

PAIRS = [(0, 1), (2, 3)]
PACKED = False  # row-packed f32r QK hangs on HW (smoke2); keep serial


def build(reps=1):
    nc = bacc.Bacc("TRN2", target_bir_lowering=False, debug=False)
    q_ext = nc.dram_tensor("q", [HPC, S, D], DT.float32, kind="ExternalInput").ap()
    k_ext = nc.dram_tensor("k", [HPC, S, D], DT.float32, kind="ExternalInput").ap()
    v_ext = nc.dram_tensor("v", [HPC, S, D], DT.float32, kind="ExternalInput").ap()
    out_ext = nc.dram_tensor("out", [HPC, S, D], DT.float32, kind="ExternalOutput").ap()

    with tile.TileContext(nc) as tc, ExitStack() as ctx:
        const = ctx.enter_context(tc.tile_pool(name="const", bufs=1))
        sb = ctx.enter_context(tc.tile_pool(name="sb", bufs=2))
        ps_sc = ctx.enter_context(tc.tile_pool(name="ps_sc", bufs=2, space="PSUM"))
        ps_out = ctx.enter_context(tc.tile_pool(name="ps_out", bufs=1, space="PSUM"))
        ps_tr = ctx.enter_context(tc.tile_pool(name="ps_tr", bufs=2, space="PSUM"))

        ident = const.tile([P, P], DT.float32)
        make_identity(nc, ident[:])

        # ---------------- per-pair state ----------------
        def load_pair(pi):
            """Chunked loads, critical-group-first. vx (bf16 [v|1]) is built
            per 4-tile chunk on GPSIMD so the DVE FIFO never blocks on the
            v-side DMAs."""
            hA, hB = PAIRS[pi]
            st = {}
            for nm in ("A", "B"):
                for pref in ("q_nat", "k_nat", "v_nat"):
                    st[pref + nm] = sb.tile(
                        [P, NO, D], DT.float32, tag=pref + nm, name=pref + nm
                    )
                st["vx" + nm] = sb.tile(
                    [P, NT, D + 1], DT.bfloat16, tag="vx" + nm, name="vx" + nm
                )
            st["pi"] = pi
            load_groups(st, [0])
            return st

        def load_groups(st, groups, parts="kqv"):
            hA, hB = PAIRS[st["pi"]]
            for g in groups:
                sl = slice(g * 4, g * 4 + 4)
                prefs = []
                if "k" in parts:
                    prefs.append(("k_nat", k_ext))
                if "q" in parts:
                    prefs.append(("q_nat", q_ext))
                for pref, ext in prefs:
                    for nm, h in (("B", hB), ("A", hA)):
                        nc.sync.dma_start(
                            st[pref + nm][:, sl, :],
                            ext[h].rearrange("(o p) d -> p o d", p=P)[:, sl, :],
                        )
                if "v" not in parts:
                    continue
                for nm, h in (("A", hA), ("B", hB)):
                    nc.sync.dma_start(
                        st["v_nat" + nm][:, sl, :],
                        v_ext[h].rearrange("(o p) d -> p o d", p=P)[:, sl, :],
                    )
                    vx = st["vx" + nm]
                    nc.gpsimd.tensor_copy(vx[:, sl, :D], st["v_nat" + nm][:, sl, :])
                    nc.gpsimd.memset(vx[:, sl, D], 1.0)

        def alloc_T(st):
            if PACKED:
                qT = sb.tile([P, NO, P], DT.float32r, tag="qT", name="qT")
                kT = sb.tile([P, NT, P], DT.float32r, tag="kT", name="kT")
                st["qT_A"], st["qT_B"] = qT[:64], qT[64:128]
                st["kT_A"], st["kT_B"] = kT[:64], kT[64:128]
            else:
                for nm in ("A", "B"):
                    st["qT_" + nm] = sb.tile(
                        [64, NO, P], DT.float32r, tag="qT" + nm, name="qT" + nm
                    )
                    st["kT_" + nm] = sb.tile(
                        [64, NT, P], DT.float32r, tag="kT" + nm, name="kT" + nm
                    )

        def emit_tr_group(st, which, nm, g):
            """Transpose 4 tiles of q/k into qT/kT; head A -> rows 0:64,
            head B -> rows 64:128 via staging + partition-shift DMA."""
            src = st[("q_nat" if which == "q" else "k_nat") + nm]
            dst = st[("qT_" if which == "q" else "kT_") + nm]
            tr_ps = ps_tr.tile([64, 4, P], DT.float32, tag="tr")
            for j in range(4):
                nc.tensor.transpose(tr_ps[:, j, :], src[:, g * 4 + j, :], ident[:])
            if nm == "A" or not PACKED:
                nc.vector.tensor_copy(dst[:, g * 4 : g * 4 + 4, :], tr_ps[:])
            else:
                stage = sb.tile([64, 4, P], DT.float32r, tag="stage")
                nc.vector.tensor_copy(stage[:], tr_ps[:])
                nc.gpsimd.dma_start(dst[:, g * 4 : g * 4 + 4, :], stage[:])

        def emit_fin(st, pi, c):
            """Finalize chunk c of pair pi: divide by denom, transpose, DMA."""
            for nm, h in (("A", PAIRS[pi][0]), ("B", PAIRS[pi][1])):
                out_ps = st.pop(("outA" if nm == "A" else "outB") + str(c))
                outT_sb = sb.tile([D + 1, CH], DT.float32, tag="outT")
                nc.vector.tensor_copy(outT_sb[:], out_ps[:])
                fin_ps = ps_tr.tile([P, 4, D + 1], DT.float32, tag="tr")
                for j in range(4):
                    nc.tensor.transpose(
                        fin_ps[:, j, :],
                        outT_sb[:, j * P : (j + 1) * P],
                        ident[: D + 1, : D + 1],
                    )
                recip = sb.tile([P, 4, 1], DT.float32, tag="recip")
                nc.vector.reciprocal(recip[:], fin_ps[:, :, D : D + 1])
                outn = sb.tile([P, 4, D], DT.float32, tag="outn")
                nc.vector.tensor_tensor(
                    outn[:],
                    fin_ps[:, :, :D],
                    recip[:].to_broadcast((P, 4, D)),
                    mybir.AluOpType.mult,
                )
                nc.sync.dma_start(
                    out_ext[h].rearrange("(o p) d -> p o d", p=P)[
                        :, c * 4 : c * 4 + 4, :
                    ],
                    outn[:],
                )

        def emit_pv(st, outA, outB, probs, t):
            nc.tensor.matmul(
                outA[:],
                st["vxA"][:, t, :],
                probs[:, :CH],
                start=(t == 0),
                stop=(t == NT - 1),
            )
            nc.tensor.matmul(
                outB[:],
                st["vxB"][:, t, :],
                probs[:, CH:],
                start=(t == 0),
                stop=(t == NT - 1),
            )

        # ---------------- emission schedule ----------------
        def emit_all():
            # prologue: pair 0 DMAs + vx + minimal transposes to start chunk 0
            st_cur = load_pair(0)
            alloc_T(st_cur)
            for which, nm in (("k", "B"), ("k", "A"), ("q", "B"), ("q", "A")):
                emit_tr_group(st_cur, which, nm, 0)

            st_next = None
            pending_fin = None  # (st, pi, c) finalization to emit inside next chunk
            pending_pv = None

            for pi in range(len(PAIRS)):
                for c in range(NCH):
                    ci = pi * NCH + c  # global chunk index 0..7
                    st = st_cur
                    outA = ps_out.tile([D + 1, CH], DT.float32, tag="outA", name="outA")
                    outB = ps_out.tile([D + 1, CH], DT.float32, tag="outB", name="outB")
                    st["outA" + str(c)] = outA
                    st["outB" + str(c)] = outB
                    for t in range(NT):
                        scp = ps_sc.tile([P, 2 * CH], DT.float32, tag="scores")
                        nc.tensor.matmul(
                            scp[:, :CH],
                            st["kT_A"][:, t, :],
                            st["qT_A"][:, c * 4 : c * 4 + 4, :],
                            start=True,
                            stop=True,
                        )
                        nc.tensor.matmul(
                            scp[:, CH:],
                            st["kT_B"][:, t, :],
                            st["qT_B"][:, c * 4 : c * 4 + 4, :],
                            start=True,
                            stop=True,
                        )
                        probs = sb.tile(
                            [P, 2 * CH], DT.bfloat16, tag="probs", name="probs"
                        )
                        nc.scalar.activation(probs[:], scp[:], AF.Exp)
                        # PV delayed one iteration: QK(t+1) enters the PE FIFO
                        # before PV(t), so the next exp is never queued behind PV.
                        if pending_pv is not None:
                            emit_pv(*pending_pv)
                        pending_pv = (st, outA, outB, probs, t)

                        # --- pipelined emissions ---
                        if t == 2 and pending_fin is not None:
                            emit_fin(*pending_fin)
                            pending_fin = None
                        if ci == 0:
                            # interleaved remainder of pair-0 setup; every DMA is
                            # emitted strictly before the transposes that read it
                            if t == 0:
                                load_groups(st, [1], parts="kv")
                            if t == 2:
                                load_groups(st, [1], parts="q")
                            if t == 3:
                                load_groups(st, [2], parts="kv")
                            if t == 6:
                                load_groups(st, [2], parts="q")
                            if t == 7:
                                load_groups(st, [3], parts="kv")
                            if t == 10:
                                load_groups(st, [3], parts="q")
                            if t in (1, 5, 9):
                                g = t // 4 + 1
                                emit_tr_group(st, "k", "B", g)
                                emit_tr_group(st, "k", "A", g)
                            if t in (3, 7, 11):
                                g = (t - 3) // 4 + 1
                                emit_tr_group(st, "q", "B", g)
                                emit_tr_group(st, "q", "A", g)
                        if ci % NCH == 1 and t == 2 and pi + 1 < len(PAIRS):
                            st_next = load_pair(pi + 1)
                            load_groups(st_next, [1, 2, 3])
                            alloc_T(st_next)
                            st_next["gi"] = 0
                        if (
                            st_next is not None
                            and st_next.get("gi", 16) < 16
                            and t % 2 == 1
                            and (ci % NCH > 1 or t >= 5)
                        ):
                            gi = st_next["gi"]
                            st_next["gi"] = gi + 1
                            which = ("k", "q")[gi // 8]
                            nm = ("A", "B")[(gi // 4) % 2]
                            emit_tr_group(st_next, which, nm, gi % 4)

                    pending_fin = (st, pi, c)
                    if ci == len(PAIRS) * NCH - 1:
                        # very last chunk: flush trailing PV + fin
                        emit_pv(*pending_pv)
                        pending_pv = None
                        emit_fin(*pending_fin)
                        pending_fin = None

                st_cur = st_next
                st_next = None

        if reps == 1:
            emit_all()
        else:
            with tc.For_i(0, reps, 1):
                emit_all()

    nc.compile()
    return nc


_NC = None


def _get_nc():
    global _NC
    if _NC is None:
        _NC = build()
    return _NC


def kernel(q: np.ndarray, k: np.ndarray, v: np.ndarray) -> np.ndarray:
    qf = np.ascontiguousarray(q, dtype=np.float32).reshape(B * H, S, D)
    kf = np.ascontiguousarray(k, dtype=np.float32).reshape(B * H, S, D)
    vf = np.ascontiguousarray(v, dtype=np.float32).reshape(B * H, S, D)
    in_maps = [
        {
            "q": qf[c * HPC : (c + 1) * HPC],
            "k": kf[c * HPC : (c + 1) * HPC],
            "v": vf[c * HPC : (c + 1) * HPC],
        }
        for c in range(NCORES)
    ]
    nc = _get_nc()
    res = run_bass_kernel_spmd(nc, in_maps, core_ids=list(range(NCORES)))
    out = np.concatenate([res.results[c]["out"] for c in range(NCORES)], axis=0)
    return out.reshape(B, H, S, D)

